# revision 10
# baseline (speedup 1.0000x reference)
"""Causal multi-head attention block (16 heads, dim 1024) on 8 TRN2 NeuronCores.

Sharding: tensor-parallel over heads — core c computes heads {2c, 2c+1}:
  q/k/v projections with the 128-column weight slices, causal attention,
  and a partial output projection with the matching 128 Wout rows.
Host sums the 8 partial outputs and adds the bias.

Per-core dataflow (per batch of 2048 tokens):
  1. x tiles [128 tok, 1024] -> PE-transpose -> xT [dim-major]
  2. qT/kT/vT = W.T @ xT  (feature-major, 2 heads on 128 partitions)
     vT -> PE-transpose -> v_aug [tok-major, 65 cols/head: 64 v + ones]
  3. scores computed TRANSPOSED: dotsT[j,i] = kT.T @ qT -> exp (no max-
     subtraction needed; values are small) -> attnT; causal mask added on
     diagonal tiles pre-exp. AV: outT[d,i] = v_aug.T @ attnT accumulated
     over j-tiles; row 64 of the psum = softmax denominators (ones trick).
  4. normalize columns by 1/denominator, output projection, DMA out.

Matmul dtype is parameterized: float32r (single-pass fp32, ~12-bit
mantissa) or bfloat16.
"""
import numpy as np
from contextlib import ExitStack, nullcontext

import concourse.bacc as bacc
import concourse.mybir as mybir
import concourse.tile as tile
import concourse.bass_utils as bass_utils
from concourse import masks

F32 = mybir.dt.float32
F32R = mybir.dt.float32r
BF16 = mybir.dt.bfloat16

B = 4            # batches
T = 2048         # tokens per batch
DIM = 1024
NT = T // 128    # token tiles per batch (16)
KT = DIM // 128  # contraction tiles (8)
NCHUNK = T // 512  # 512-col i-chunks per batch (4)
SCALE = DIM ** -0.5  # 1/32 — NOTE: full dim, not head dim (matches reference)
MASK_NEG = -1.0e9

DEFAULT_MMDT = "f32r"
_CACHED = {}


def build_kernel(repeat=None, mmdt=None, nbatches=None, skip=()):
    mmdt = mmdt or DEFAULT_MMDT
    MMDT = {"f32r": F32R, "bf16": BF16}[mmdt]
    # element factor when bitcasting an f32 psum tile view to MMDT
    EF = 1 if MMDT == F32R else 2

    nc = bacc.Bacc("TRN2", target_bir_lowering=False, debug=False, num_devices=8)

    xdt = F32R if MMDT == F32R else F32
    x_d = nc.dram_tensor("x", [B * T, DIM], xdt, kind="ExternalInput").ap()
    wq_d = nc.dram_tensor("wq", [DIM, 128], xdt, kind="ExternalInput").ap()
    wk_d = nc.dram_tensor("wk", [DIM, 128], xdt, kind="ExternalInput").ap()
    wv_d = nc.dram_tensor("wv", [DIM, 128], xdt, kind="ExternalInput").ap()
    wo_d = nc.dram_tensor("wo", [128, DIM], xdt, kind="ExternalInput").ap()
    out_d = nc.dram_tensor("out", [B * T, DIM], F32, kind="ExternalOutput").ap()

    with tile.TileContext(nc) as tc, ExitStack() as ctx:
        cp = ctx.enter_context(tc.tile_pool(name="const", bufs=1))
        xin_p = ctx.enter_context(tc.tile_pool(name="xin", bufs=2))
        xT_p = ctx.enter_context(tc.tile_pool(name="xT", bufs=1))
        qT_p = ctx.enter_context(tc.tile_pool(name="qT", bufs=2))
        kT_p = ctx.enter_context(tc.tile_pool(name="kT", bufs=2))
        vT_p = ctx.enter_context(tc.tile_pool(name="vT", bufs=1))
        vaug_p = ctx.enter_context(tc.tile_pool(name="vaug", bufs=2))
        attnT_p = ctx.enter_context(tc.tile_pool(name="attnT", bufs=4))
        recip_p = ctx.enter_context(tc.tile_pool(name="recip", bufs=2))
        rbc_p = ctx.enter_context(tc.tile_pool(name="rbc", bufs=2))
        outT_p = ctx.enter_context(tc.tile_pool(name="outT", bufs=1))
        osb_p = ctx.enter_context(tc.tile_pool(name="osb", bufs=2))
        mm_ps = ctx.enter_context(tc.tile_pool(name="mmps", bufs=2, space="PSUM"))
        dots_ps = ctx.enter_context(tc.tile_pool(name="dotsps", bufs=4, space="PSUM"))
        av_ps_p = ctx.enter_context(tc.tile_pool(name="avps", bufs=2, space="PSUM"))

        # ---- constants ----
        ident32 = cp.tile([128, 128], F32, tag="ident32")
        masks.make_identity(nc, ident32[:])
        ident = cp.tile([128, 128], MMDT, tag="ident")
        nc.vector.tensor_copy(ident[:], ident32[:])

        # additive causal mask for a diagonal 128x128 tile in [j, i] layout:
        # 0 where j <= i, MASK_NEG where j > i.
        maskT32 = cp.tile([128, 128], F32, tag="maskT32")
        nc.gpsimd.memset(maskT32[:], 0.0)
        nc.gpsimd.affine_select(
            out=maskT32[:], in_=maskT32[:],
            compare_op=mybir.AluOpType.is_ge, fill=MASK_NEG,
            base=0, pattern=[[-1, 128]], channel_multiplier=1,
        )
        maskT = cp.tile([128, 128], MMDT, tag="maskT")
        nc.vector.tensor_copy(maskT[:], maskT32[:])

        ones32 = cp.tile([128, 2 * NT], F32, tag="ones32")
        nc.gpsimd.memset(ones32[:], 1.0)

        drain_flip = [0]
        def drain_copy(dst, src_ap):
            # alternate PSUM->SBUF drains between DVE and ACT so neither
            # engine's copy rate paces the PE
            if drain_flip[0] % 2 == 0:
                nc.vector.tensor_copy(dst, src_ap)
            else:
                nc.scalar.copy(dst, src_ap)
            drain_flip[0] += 1

        # ---- weights ----
        wq_sb = cp.tile([128, KT * 128], MMDT, tag="wq")
        wk_sb = cp.tile([128, KT * 128], MMDT, tag="wk")
        wv_sb = cp.tile([128, KT * 128], MMDT, tag="wv")
        wo_sb = cp.tile([128, DIM], MMDT, tag="wo")
        if MMDT == F32R:
            for w_sb, w_d in ((wq_sb, wq_d), (wk_sb, wk_d), (wv_sb, wv_d)):
                nc.sync.dma_start(w_sb[:].rearrange("p (kt m) -> p kt m", kt=KT),
                                  w_d.rearrange("(kt p) m -> p kt m", p=128))
            nc.sync.dma_start(wo_sb[:], wo_d)
        else:
            for w_sb, w_d, wtag in ((wq_sb, wq_d, "q"), (wk_sb, wk_d, "k"),
                                    (wv_sb, wv_d, "v")):
                w32 = cp.tile([128, KT * 128], F32, tag=f"w32{wtag}", name=f"w32{wtag}")
                nc.sync.dma_start(w32[:].rearrange("p (kt m) -> p kt m", kt=KT),
                                  w_d.rearrange("(kt p) m -> p kt m", p=128))
                nc.vector.tensor_copy(w_sb[:], w32[:])
            wo32 = cp.tile([128, DIM], F32, tag="wo32")
            nc.sync.dma_start(wo32[:], wo_d)
            nc.vector.tensor_copy(wo_sb[:], wo32[:])

        rep_ctx = tc.For_i(0, repeat, 1) if repeat is not None else nullcontext()
        with rep_ctx:
         for b in range(nbatches if nbatches is not None else B):
             t0 = b * T
             # ---- phase 1: xT [128, KT*T] (dim-major), via PE transposes ----
             xT = xT_p.tile([128, KT * T], MMDT, tag="xT")
             if "xT" in skip:
                 nc.gpsimd.memset(xT[:], 0.01)
             for tt in (() if "xT" in skip else range(NT)):
                 xin = xin_p.tile([128, DIM], xdt, tag="xin")
                 nc.sync.dma_start(xin[:], x_d[t0 + tt * 128: t0 + (tt + 1) * 128, :])
                 if MMDT == F32R:
                     xsrc = xin
                 else:
                     xsrc = xin_p.tile([128, DIM], BF16, tag="xinb", name="xinb")
                     nc.gpsimd.tensor_copy(xsrc[:], xin[:])
                 for ktg in range(KT // 4):  # groups of 4 transposes per psum tile
                     tp = mm_ps.tile([128, 512], F32, tag="mm")
                     tpv = tp[:].bitcast(MMDT)
                     for j in range(4):
                         kt = 4 * ktg + j
                         nc.tensor.transpose(tpv[:, j * 128:(j + 1) * 128],
                                             xsrc[:, kt * 128:(kt + 1) * 128], ident[:])
                     dst = xT[:].rearrange("p (kt t) -> p kt t", kt=KT)[
                         :, 4 * ktg:4 * ktg + 4, tt * 128:(tt + 1) * 128]
                     drain_copy(dst, tpv[:, 0:512].rearrange("p (j c) -> p j c", j=4))

             # ---- phase 2: projections qT/kT/vT [128, T] (feature-major) ----
             qkv = []
             for w_sb, pool, tag in ((wq_sb, qT_p, "qT"), (wk_sb, kT_p, "kT"),
                                     (wv_sb, vT_p, "vT")):
                 dest = pool.tile([128, T], MMDT, tag=tag)
                 qkv.append(dest)
                 if "proj" in skip:
                     nc.gpsimd.memset(dest[:], 0.01)
                     continue
                 for ch in range(NCHUNK):
                     pp = mm_ps.tile([128, 512], F32, tag="mm")
                     for kt in range(KT):
                         nc.tensor.matmul(
                             pp[:], w_sb[:, kt * 128:(kt + 1) * 128],
                             xT[:, kt * T + ch * 512: kt * T + (ch + 1) * 512],
                             start=(kt == 0), stop=(kt == KT - 1))
                     drain_copy(dest[:, ch * 512:(ch + 1) * 512], pp[:])
             qT, kT_t, vT = qkv

             # ---- phase 2b: v_aug [128 tok, NT * 130] via PE transposes of vT ----
             vaug = vaug_p.tile([128, NT * 130], MMDT, tag="vaug")
             if "proj" in skip:
                 nc.gpsimd.memset(vaug[:], 1.0)
             for jtg in (() if "proj" in skip else range(NT // 4)):
                 tp = mm_ps.tile([128, 512], F32, tag="mm")
                 tpv = tp[:].bitcast(MMDT)
                 for j in range(4):
                     jt = 4 * jtg + j
                     nc.tensor.transpose(tpv[:, j * 128:(j + 1) * 128],
                                         vT[:, jt * 128:(jt + 1) * 128], ident[:])
                 vv = vaug[:].rearrange("p (jt c) -> p jt c", c=130)
                 src = tpv[:, 0:512].rearrange("p (j c) -> p j c", j=4)
                 nc.vector.tensor_copy(vv[:, 4 * jtg:4 * jtg + 4, 0:64], src[:, :, 0:64])
                 nc.vector.tensor_copy(vv[:, 4 * jtg:4 * jtg + 4, 65:129], src[:, :, 64:128])
             # ones columns at 64 and 129 of each 130-block: stride-65 pattern
             if "proj" not in skip:
                 nc.vector.tensor_copy(
                     vaug[:].rearrange("p (u c) -> p u c", c=65)[:, :, 64:65],
                     ones32[:].rearrange("p (u o) -> p u o", o=1))

             # ---- phase 3: attention (both heads), outT [128, T] ----
             outT = outT_p.tile([128, T], MMDT, tag="outT")
             if "attn" in skip:
                 nc.gpsimd.memset(outT[:], 0.01)
             for c in (() if "attn" in skip else range(NCHUNK)):
                 njt = 4 * (c + 1)  # j-tiles touching this i-chunk
                 avp = {h: av_ps_p.tile([65, 512], F32, tag="av", name=f"avp{h}") for h in (0, 1)}
                 for jt in range(njt):
                     istart = max(512 * c, jt * 128)
                     off = istart - 512 * c
                     diag = jt >= 4 * c
                     dps, ats = {}, {}
                     for h in (0, 1):
                         dp = dots_ps.tile([128, 512], F32, tag="dots", name=f"dp{h}")
                         dps[h] = dp
                         nc.tensor.matmul(
                             dp[:, off:512],
                             kT_t[64 * h:64 * h + 64, jt * 128:(jt + 1) * 128],
                             qT[64 * h:64 * h + 64, istart:512 * (c + 1)],
                             start=True, stop=not diag)
                         if diag:  # causal mask added via PE: maskT.T @ I
                             nc.tensor.matmul(
                                 dp[:, off:off + 128], maskT[:], ident[:],
                                 start=False, stop=True)
                     for h in (0, 1):
                         at = attnT_p.tile([128, 512], MMDT, tag="at", name=f"at{h}")
                         nc.scalar.activation(at[:, off:512], dps[h][:, off:512],
                                              mybir.ActivationFunctionType.Exp,
                                              bias=0.0, scale=float(SCALE))
                         ats[h] = at
                     for h in (0, 1):
                         nc.tensor.matmul(
                             avp[h][:, off:512],
                             vaug[:, jt * 130 + 65 * h: jt * 130 + 65 * h + 65],
                             ats[h][:, off:512],
                             start=(jt == 0), stop=(jt == njt - 1))
                 for h in (0, 1):
                     rc = recip_p.tile([1, 512], F32, tag="recip")
                     nc.vector.reciprocal(rc[:], avp[h][64:65, :])
                     rb = rbc_p.tile([64, 512], F32, tag="rbc")
                     nc.gpsimd.partition_broadcast(rb[:], rc[:])
                     nc.vector.scalar_tensor_tensor(
                         outT[64 * h:64 * h + 64, c * 512:(c + 1) * 512],
                         avp[h][0:64, :], 1.0, rb[:],
                         op0=mybir.AluOpType.mult, op1=mybir.AluOpType.mult)

             # ---- phase 4: output projection ----
             for tt in (() if "outproj" in skip else range(NT)):
                 osb = osb_p.tile([128, DIM], F32, tag="osb")
                 for half in (0, 1):
                     po = mm_ps.tile([128, 512], F32, tag="mm")
                     nc.tensor.matmul(po[:], outT[:, tt * 128:(tt + 1) * 128],
                                      wo_sb[:, half * 512:(half + 1) * 512],
                                      start=True, stop=True)
                     drain_copy(osb[:, half * 512:(half + 1) * 512], po[:])
                 nc.sync.dma_start(out_d[t0 + tt * 128: t0 + (tt + 1) * 128, :], osb[:])

    nc.compile()
    return nc


def kernel(x, Wq, Wkv, Wout, bout):
    """Full inputs -> full output. Shards across 8 NeuronCores internally."""
    if "nc" not in _CACHED:
        _CACHED["nc"] = build_kernel()
    nc = _CACHED["nc"]

    x = np.ascontiguousarray(np.asarray(x, dtype=np.float32).reshape(B * T, DIM))
    Wq = np.asarray(Wq, dtype=np.float32)
    Wkv = np.asarray(Wkv, dtype=np.float32)
    Wout = np.asarray(Wout, dtype=np.float32)
    bout = np.asarray(bout, dtype=np.float32)

    in_maps = []
    for c in range(8):
        s = slice(128 * c, 128 * (c + 1))
        in_maps.append({
            "x": x,
            "wq": np.ascontiguousarray(Wq[:, s]),
            "wk": np.ascontiguousarray(Wkv[:, :DIM][:, s]),
            "wv": np.ascontiguousarray(Wkv[:, DIM:][:, s]),
            "wo": np.ascontiguousarray(Wout[s, :]),
        })

    res = bass_utils.run_bass_kernel_spmd(nc, in_maps, core_ids=list(range(8)))
    acc = res.results[0]["out"].astype(np.float64)
    for c in range(1, 8):
        acc += res.results[c]["out"]
    out = (acc + bout.astype(np.float64)).astype(np.float32)
    return out.reshape(B, T, DIM)


# revision 11
# speedup vs baseline: 1.0919x; 1.0919x over previous
"""Causal multi-head attention block (16 heads, dim 1024) on 8 TRN2 NeuronCores.

Sharding: tensor-parallel over heads — core c computes heads {2c, 2c+1}:
  q/k/v projections with the 128-column weight slices, causal attention,
  and a partial output projection with the matching 128 Wout rows.
Host sums the 8 partial outputs and adds the bias.

Per-core dataflow (per batch of 2048 tokens):
  1. x tiles [128 tok, 1024] -> PE-transpose -> xT [dim-major]
  2. qT/kT/vT = W.T @ xT  (feature-major, 2 heads on 128 partitions)
     vT -> PE-transpose -> v_aug [tok-major, 65 cols/head: 64 v + ones]
  3. scores computed TRANSPOSED: dotsT[j,i] = kT.T @ qT -> exp (no max-
     subtraction needed; values are small) -> attnT; causal mask added on
     diagonal tiles pre-exp. AV: outT[d,i] = v_aug.T @ attnT accumulated
     over j-tiles; row 64 of the psum = softmax denominators (ones trick).
  4. normalize columns by 1/denominator, output projection, DMA out.

Matmul dtype is parameterized: float32r (single-pass fp32, ~12-bit
mantissa) or bfloat16.
"""
import numpy as np
from contextlib import ExitStack, nullcontext

import concourse.bacc as bacc
import concourse.mybir as mybir
import concourse.tile as tile
import concourse.bass_utils as bass_utils
from concourse import masks

F32 = mybir.dt.float32
F32R = mybir.dt.float32r
BF16 = mybir.dt.bfloat16

B = 4            # batches
T = 2048         # tokens per batch
DIM = 1024
NT = T // 128    # token tiles per batch (16)
KT = DIM // 128  # contraction tiles (8)
NCHUNK = T // 512  # 512-col i-chunks per batch (4)
SCALE = DIM ** -0.5  # 1/32 — NOTE: full dim, not head dim (matches reference)
MASK_NEG = -1.0e9

DEFAULT_MMDT = "bf16"
_CACHED = {}


def build_kernel(repeat=None, mmdt=None, nbatches=None, skip=()):
    mmdt = mmdt or DEFAULT_MMDT
    MMDT = {"f32r": F32R, "bf16": BF16}[mmdt]
    # element factor when bitcasting an f32 psum tile view to MMDT
    EF = 1 if MMDT == F32R else 2

    nc = bacc.Bacc("TRN2", target_bir_lowering=False, debug=False, num_devices=8)

    xdt = F32R if MMDT == F32R else F32
    x_d = nc.dram_tensor("x", [B * T, DIM], xdt, kind="ExternalInput").ap()
    wq_d = nc.dram_tensor("wq", [DIM, 128], xdt, kind="ExternalInput").ap()
    wk_d = nc.dram_tensor("wk", [DIM, 128], xdt, kind="ExternalInput").ap()
    wv_d = nc.dram_tensor("wv", [DIM, 128], xdt, kind="ExternalInput").ap()
    wo_d = nc.dram_tensor("wo", [128, DIM], xdt, kind="ExternalInput").ap()
    out_d = nc.dram_tensor("out", [B * T, DIM], F32, kind="ExternalOutput").ap()

    with tile.TileContext(nc) as tc, ExitStack() as ctx:
        cp = ctx.enter_context(tc.tile_pool(name="const", bufs=1))
        xin_p = ctx.enter_context(tc.tile_pool(name="xin", bufs=4))
        xT_p = ctx.enter_context(tc.tile_pool(name="xT", bufs=1))
        qT_p = ctx.enter_context(tc.tile_pool(name="qT", bufs=2))
        kT_p = ctx.enter_context(tc.tile_pool(name="kT", bufs=2))
        vT_p = ctx.enter_context(tc.tile_pool(name="vT", bufs=1))
        vaug_p = ctx.enter_context(tc.tile_pool(name="vaug", bufs=2))
        attnT_p = ctx.enter_context(tc.tile_pool(name="attnT", bufs=4))
        recip_p = ctx.enter_context(tc.tile_pool(name="recip", bufs=2))
        rbc_p = ctx.enter_context(tc.tile_pool(name="rbc", bufs=2))
        outT_p = ctx.enter_context(tc.tile_pool(name="outT", bufs=1))
        osb_p = ctx.enter_context(tc.tile_pool(name="osb", bufs=3))
        mm_ps = ctx.enter_context(tc.tile_pool(name="mmps", bufs=2, space="PSUM"))
        dots_ps = ctx.enter_context(tc.tile_pool(name="dotsps", bufs=4, space="PSUM"))
        av_ps_p = ctx.enter_context(tc.tile_pool(name="avps", bufs=2, space="PSUM"))

        # ---- constants ----
        ident32 = cp.tile([128, 128], F32, tag="ident32")
        masks.make_identity(nc, ident32[:])
        ident = cp.tile([128, 128], MMDT, tag="ident")
        nc.vector.tensor_copy(ident[:], ident32[:])

        # additive causal mask for a diagonal 128x128 tile in [j, i] layout:
        # 0 where j <= i, MASK_NEG where j > i.
        maskT32 = cp.tile([128, 128], F32, tag="maskT32")
        nc.gpsimd.memset(maskT32[:], 0.0)
        nc.gpsimd.affine_select(
            out=maskT32[:], in_=maskT32[:],
            compare_op=mybir.AluOpType.is_ge, fill=MASK_NEG,
            base=0, pattern=[[-1, 128]], channel_multiplier=1,
        )
        maskT = cp.tile([128, 128], MMDT, tag="maskT")
        nc.vector.tensor_copy(maskT[:], maskT32[:])

        ones32 = cp.tile([128, 2 * NT], F32, tag="ones32")
        nc.gpsimd.memset(ones32[:], 1.0)

        def drain_copy(dst, src_ap):
            # PSUM->SBUF drains stay off ACT (ACT is saturated by exp)
            nc.vector.tensor_copy(dst, src_ap)

        # ---- weights ----
        wq_sb = cp.tile([128, KT * 128], MMDT, tag="wq")
        wk_sb = cp.tile([128, KT * 128], MMDT, tag="wk")
        wv_sb = cp.tile([128, KT * 128], MMDT, tag="wv")
        wo_sb = cp.tile([128, DIM], MMDT, tag="wo")
        if MMDT == F32R:
            for w_sb, w_d in ((wq_sb, wq_d), (wk_sb, wk_d), (wv_sb, wv_d)):
                nc.sync.dma_start(w_sb[:].rearrange("p (kt m) -> p kt m", kt=KT),
                                  w_d.rearrange("(kt p) m -> p kt m", p=128))
            nc.sync.dma_start(wo_sb[:], wo_d)
        else:
            for w_sb, w_d, wtag in ((wq_sb, wq_d, "q"), (wk_sb, wk_d, "k"),
                                    (wv_sb, wv_d, "v")):
                w32 = cp.tile([128, KT * 128], F32, tag=f"w32{wtag}", name=f"w32{wtag}")
                nc.sync.dma_start(w32[:].rearrange("p (kt m) -> p kt m", kt=KT),
                                  w_d.rearrange("(kt p) m -> p kt m", p=128))
                nc.vector.tensor_copy(w_sb[:], w32[:])
            wo32 = cp.tile([128, DIM], F32, tag="wo32")
            nc.sync.dma_start(wo32[:], wo_d)
            nc.vector.tensor_copy(wo_sb[:], wo32[:])

        rep_ctx = tc.For_i(0, repeat, 1) if repeat is not None else nullcontext()
        with rep_ctx:
         for b in range(nbatches if nbatches is not None else B):
             t0 = b * T
             # ---- phase 1: xT [128, KT*T] (dim-major), via PE transposes ----
             xT = xT_p.tile([128, KT * T], MMDT, tag="xT")
             if "xT" in skip:
                 nc.gpsimd.memset(xT[:], 0.01)
             for tt in (() if "xT" in skip else range(NT)):
                 xin = xin_p.tile([128, DIM], xdt, tag="xin")
                 nc.sync.dma_start(xin[:], x_d[t0 + tt * 128: t0 + (tt + 1) * 128, :])
                 if MMDT == F32R:
                     xsrc = xin
                 else:
                     xsrc = xin_p.tile([128, DIM], BF16, tag="xinb", name="xinb")
                     nc.gpsimd.tensor_copy(xsrc[:], xin[:])
                 for ktg in range(KT // 4):  # groups of 4 transposes per psum tile
                     tp = mm_ps.tile([128, 512], F32, tag="mm")
                     tpv = tp[:].bitcast(MMDT)
                     for j in range(4):
                         kt = 4 * ktg + j
                         nc.tensor.transpose(tpv[:, j * 128:(j + 1) * 128],
                                             xsrc[:, kt * 128:(kt + 1) * 128], ident[:])
                     dst = xT[:].rearrange("p (kt t) -> p kt t", kt=KT)[
                         :, 4 * ktg:4 * ktg + 4, tt * 128:(tt + 1) * 128]
                     drain_copy(dst, tpv[:, 0:512].rearrange("p (j c) -> p j c", j=4))

             # ---- phase 2: projections qT/kT/vT [128, T] (feature-major) ----
             qkv = []
             for w_sb, pool, tag in ((wq_sb, qT_p, "qT"), (wk_sb, kT_p, "kT"),
                                     (wv_sb, vT_p, "vT")):
                 dest = pool.tile([128, T], MMDT, tag=tag)
                 qkv.append(dest)
                 if "proj" in skip:
                     nc.gpsimd.memset(dest[:], 0.01)
                     continue
                 for ch in range(NCHUNK):
                     pp = mm_ps.tile([128, 512], F32, tag="mm")
                     for kt in range(KT):
                         nc.tensor.matmul(
                             pp[:], w_sb[:, kt * 128:(kt + 1) * 128],
                             xT[:, kt * T + ch * 512: kt * T + (ch + 1) * 512],
                             start=(kt == 0), stop=(kt == KT - 1))
                     drain_copy(dest[:, ch * 512:(ch + 1) * 512], pp[:])
             qT, kT_t, vT = qkv

             # ---- phase 2b: v_aug [128 tok, NT * 130] via PE transposes of vT ----
             vaug = vaug_p.tile([128, NT * 130], MMDT, tag="vaug")
             if "proj" in skip:
                 nc.gpsimd.memset(vaug[:], 1.0)
             for jtg in (() if "proj" in skip else range(NT // 4)):
                 tp = mm_ps.tile([128, 512], F32, tag="mm")
                 tpv = tp[:].bitcast(MMDT)
                 for j in range(4):
                     jt = 4 * jtg + j
                     nc.tensor.transpose(tpv[:, j * 128:(j + 1) * 128],
                                         vT[:, jt * 128:(jt + 1) * 128], ident[:])
                 vv = vaug[:].rearrange("p (jt c) -> p jt c", c=130)
                 src = tpv[:, 0:512].rearrange("p (j c) -> p j c", j=4)
                 nc.vector.tensor_copy(vv[:, 4 * jtg:4 * jtg + 4, 0:64], src[:, :, 0:64])
                 nc.vector.tensor_copy(vv[:, 4 * jtg:4 * jtg + 4, 65:129], src[:, :, 64:128])
             # ones columns at 64 and 129 of each 130-block: stride-65 pattern
             if "proj" not in skip:
                 nc.vector.tensor_copy(
                     vaug[:].rearrange("p (u c) -> p u c", c=65)[:, :, 64:65],
                     ones32[:].rearrange("p (u o) -> p u o", o=1))

             # ---- phase 3: attention (both heads), outT [128, T] ----
             outT = outT_p.tile([128, T], MMDT, tag="outT")
             if "attn" in skip:
                 nc.gpsimd.memset(outT[:], 0.01)
             for c in (() if "attn" in skip else range(NCHUNK)):
                 njt = 4 * (c + 1)  # j-tiles touching this i-chunk
                 avp = {h: av_ps_p.tile([65, 512], F32, tag="av", name=f"avp{h}") for h in (0, 1)}
                 for jt in range(njt):
                     istart = max(512 * c, jt * 128)
                     off = istart - 512 * c
                     diag = jt >= 4 * c
                     dps, ats = {}, {}
                     for h in (0, 1):
                         dp = dots_ps.tile([128, 512], F32, tag="dots", name=f"dp{h}")
                         dps[h] = dp
                         nc.tensor.matmul(
                             dp[:, off:512],
                             kT_t[64 * h:64 * h + 64, jt * 128:(jt + 1) * 128],
                             qT[64 * h:64 * h + 64, istart:512 * (c + 1)],
                             start=True, stop=not diag)
                         if diag:  # causal mask added via PE: maskT.T @ I
                             nc.tensor.matmul(
                                 dp[:, off:off + 128], maskT[:], ident[:],
                                 start=False, stop=True)
                     for h in (0, 1):
                         at = attnT_p.tile([128, 512], MMDT, tag="at", name=f"at{h}")
                         nc.scalar.activation(at[:, off:512], dps[h][:, off:512],
                                              mybir.ActivationFunctionType.Exp,
                                              bias=0.0, scale=float(SCALE))
                         ats[h] = at
                     for h in (0, 1):
                         nc.tensor.matmul(
                             avp[h][:, off:512],
                             vaug[:, jt * 130 + 65 * h: jt * 130 + 65 * h + 65],
                             ats[h][:, off:512],
                             start=(jt == 0), stop=(jt == njt - 1))
                 for h in (0, 1):
                     rc = recip_p.tile([1, 512], F32, tag="recip")
                     nc.vector.reciprocal(rc[:], avp[h][64:65, :])
                     rb = rbc_p.tile([64, 512], F32, tag="rbc")
                     nc.gpsimd.partition_broadcast(rb[:], rc[:])
                     nc.vector.scalar_tensor_tensor(
                         outT[64 * h:64 * h + 64, c * 512:(c + 1) * 512],
                         avp[h][0:64, :], 1.0, rb[:],
                         op0=mybir.AluOpType.mult, op1=mybir.AluOpType.mult)

             # ---- phase 4: output projection ----
             for tt in (() if "outproj" in skip else range(NT)):
                 osb = osb_p.tile([128, DIM], F32, tag="osb")
                 for half in (0, 1):
                     po = mm_ps.tile([128, 512], F32, tag="mm")
                     nc.tensor.matmul(po[:], outT[:, tt * 128:(tt + 1) * 128],
                                      wo_sb[:, half * 512:(half + 1) * 512],
                                      start=True, stop=True)
                     drain_copy(osb[:, half * 512:(half + 1) * 512], po[:])
                 nc.scalar.dma_start(out_d[t0 + tt * 128: t0 + (tt + 1) * 128, :], osb[:])

    nc.compile()
    return nc


def kernel(x, Wq, Wkv, Wout, bout):
    """Full inputs -> full output. Shards across 8 NeuronCores internally."""
    if "nc" not in _CACHED:
        _CACHED["nc"] = build_kernel()
    nc = _CACHED["nc"]

    x = np.ascontiguousarray(np.asarray(x, dtype=np.float32).reshape(B * T, DIM))
    Wq = np.asarray(Wq, dtype=np.float32)
    Wkv = np.asarray(Wkv, dtype=np.float32)
    Wout = np.asarray(Wout, dtype=np.float32)
    bout = np.asarray(bout, dtype=np.float32)

    in_maps = []
    for c in range(8):
        s = slice(128 * c, 128 * (c + 1))
        in_maps.append({
            "x": x,
            "wq": np.ascontiguousarray(Wq[:, s]),
            "wk": np.ascontiguousarray(Wkv[:, :DIM][:, s]),
            "wv": np.ascontiguousarray(Wkv[:, DIM:][:, s]),
            "wo": np.ascontiguousarray(Wout[s, :]),
        })

    res = bass_utils.run_bass_kernel_spmd(nc, in_maps, core_ids=list(range(8)))
    acc = res.results[0]["out"].astype(np.float64)
    for c in range(1, 8):
        acc += res.results[c]["out"]
    out = (acc + bout.astype(np.float64)).astype(np.float32)
    return out.reshape(B, T, DIM)


# revision 12
# speedup vs baseline: 1.3460x; 1.2326x over previous
"""Causal multi-head attention block (16 heads, dim 1024) on 8 TRN2 NeuronCores.

Sharding: tensor-parallel over heads — core c computes heads {2c, 2c+1}:
  q/k/v projections with the 128-column weight slices, causal attention,
  and a partial output projection with the matching 128 Wout rows.
Host sums the 8 partial outputs and adds the bias.

Per-core dataflow (per batch of 2048 tokens):
  phase12: x tiles -> PE-transpose -> xT (dim-major); qT/kT/vT = W.T @ xT
           (feature-major, 2 heads packed on 128 partitions); vT ->
           PE-transpose -> v_aug (tok-major, 65 cols/head: 64 v + ones).
  phase34: scores TRANSPOSED: dotsT[j,i] = kT.T @ qT, causal mask added
           via a second matmul in the same accumulation group, exp on ACT
           (no max-subtraction needed; exponents are small) -> attnT;
           AV: outT = v_aug.T @ attnT accumulated over j-tiles; psum row 64
           = softmax denominators (ones-column trick). Normalize, then
           output projection per 128-token tile, DMA out.

Engines run their instruction streams IN ORDER, so phase12(b+1) emission is
interleaved with phase34(b) to fill PE gaps left by exp latency and to keep
DMA/DVE/ACT busy concurrently (software pipelining at emission order).
"""
import numpy as np
from contextlib import ExitStack, nullcontext

import concourse.bacc as bacc
import concourse.mybir as mybir
import concourse.tile as tile
import concourse.bass_utils as bass_utils
from concourse import masks

F32 = mybir.dt.float32
F32R = mybir.dt.float32r
BF16 = mybir.dt.bfloat16

B = 4            # batches
T = 2048         # tokens per batch
DIM = 1024
NT = T // 128    # token tiles per batch (16)
KT = DIM // 128  # contraction tiles (8)
NCHUNK = T // 512  # 512-col i-chunks per batch (4)
SCALE = DIM ** -0.5  # 1/32 — NOTE: full dim, not head dim (matches reference)
MASK_NEG = -1.0e9

DEFAULT_MMDT = "bf16"
_CACHED = {}


def build_kernel(repeat=None, mmdt=None, nbatches=None, interleave=True):
    mmdt = mmdt or DEFAULT_MMDT
    MMDT = {"f32r": F32R, "bf16": BF16}[mmdt]
    NB = nbatches if nbatches is not None else B

    nc = bacc.Bacc("TRN2", target_bir_lowering=False, debug=False, num_devices=8)

    xdt = F32R if MMDT == F32R else F32
    x_d = nc.dram_tensor("x", [B * T, DIM], xdt, kind="ExternalInput").ap()
    wq_d = nc.dram_tensor("wq", [DIM, 128], xdt, kind="ExternalInput").ap()
    wk_d = nc.dram_tensor("wk", [DIM, 128], xdt, kind="ExternalInput").ap()
    wv_d = nc.dram_tensor("wv", [DIM, 128], xdt, kind="ExternalInput").ap()
    wo_d = nc.dram_tensor("wo", [128, DIM], xdt, kind="ExternalInput").ap()
    out_d = nc.dram_tensor("out", [B * T, DIM], F32, kind="ExternalOutput").ap()

    with tile.TileContext(nc) as tc, ExitStack() as ctx:
        cp = ctx.enter_context(tc.tile_pool(name="const", bufs=1))
        xin_p = ctx.enter_context(tc.tile_pool(name="xin", bufs=4))
        xT_p = ctx.enter_context(tc.tile_pool(name="xT", bufs=2))
        qT_p = ctx.enter_context(tc.tile_pool(name="qT", bufs=2))
        kT_p = ctx.enter_context(tc.tile_pool(name="kT", bufs=2))
        vT_p = ctx.enter_context(tc.tile_pool(name="vT", bufs=2))
        vaug_p = ctx.enter_context(tc.tile_pool(name="vaug", bufs=2))
        attnT_p = ctx.enter_context(tc.tile_pool(name="attnT", bufs=6))
        recip_p = ctx.enter_context(tc.tile_pool(name="recip", bufs=2))
        rbc_p = ctx.enter_context(tc.tile_pool(name="rbc", bufs=2))
        outT_p = ctx.enter_context(tc.tile_pool(name="outT", bufs=2))
        osb_p = ctx.enter_context(tc.tile_pool(name="osb", bufs=3))
        mm_ps = ctx.enter_context(tc.tile_pool(name="mmps", bufs=2, space="PSUM"))
        dots_ps = ctx.enter_context(tc.tile_pool(name="dotsps", bufs=4, space="PSUM"))
        av_ps_p = ctx.enter_context(tc.tile_pool(name="avps", bufs=2, space="PSUM"))

        # ---- constants ----
        ident32 = cp.tile([128, 128], F32, tag="ident32")
        masks.make_identity(nc, ident32[:])
        ident = cp.tile([128, 128], MMDT, tag="ident")
        nc.vector.tensor_copy(ident[:], ident32[:])

        # maskT[i, j] = MASK_NEG where j > i (applied to dotsT via maskT.T @ I)
        maskT32 = cp.tile([128, 128], F32, tag="maskT32")
        nc.gpsimd.memset(maskT32[:], 0.0)
        nc.gpsimd.affine_select(
            out=maskT32[:], in_=maskT32[:],
            compare_op=mybir.AluOpType.is_ge, fill=MASK_NEG,
            base=0, pattern=[[-1, 128]], channel_multiplier=1,
        )
        maskT = cp.tile([128, 128], MMDT, tag="maskT")
        nc.vector.tensor_copy(maskT[:], maskT32[:])

        ones32 = cp.tile([128, 2 * NT], F32, tag="ones32")
        nc.gpsimd.memset(ones32[:], 1.0)

        # ---- weights ----
        wq_sb = cp.tile([128, KT * 128], MMDT, tag="wq")
        wk_sb = cp.tile([128, KT * 128], MMDT, tag="wk")
        wv_sb = cp.tile([128, KT * 128], MMDT, tag="wv")
        wo_sb = cp.tile([128, DIM], MMDT, tag="wo")
        if MMDT == F32R:
            for w_sb, w_d in ((wq_sb, wq_d), (wk_sb, wk_d), (wv_sb, wv_d)):
                nc.sync.dma_start(w_sb[:].rearrange("p (kt m) -> p kt m", kt=KT),
                                  w_d.rearrange("(kt p) m -> p kt m", p=128))
            nc.sync.dma_start(wo_sb[:], wo_d)
        else:
            for w_sb, w_d, wtag in ((wq_sb, wq_d, "q"), (wk_sb, wk_d, "k"),
                                    (wv_sb, wv_d, "v")):
                w32 = cp.tile([128, KT * 128], F32, tag=f"w32{wtag}", name=f"w32{wtag}")
                nc.sync.dma_start(w32[:].rearrange("p (kt m) -> p kt m", kt=KT),
                                  w_d.rearrange("(kt p) m -> p kt m", p=128))
                nc.vector.tensor_copy(w_sb[:], w32[:])
            wo32 = cp.tile([128, DIM], F32, tag="wo32")
            nc.sync.dma_start(wo32[:], wo_d)
            nc.vector.tensor_copy(wo_sb[:], wo32[:])

        state = {}  # per-batch qT/kT/vaug handles

        def phase12_steps(b):
            """xT + projections + v_aug for batch b. Yields between steps."""
            t0 = b * T
            xT = xT_p.tile([128, KT * T], MMDT, tag="xT", name="xT")
            for tt in range(NT):
                xin = xin_p.tile([128, DIM], xdt, tag="xin", name="xin")
                nc.sync.dma_start(xin[:], x_d[t0 + tt * 128: t0 + (tt + 1) * 128, :])
                if MMDT == F32R:
                    xsrc = xin
                else:
                    xsrc = xin_p.tile([128, DIM], BF16, tag="xinb", name="xinb")
                    nc.gpsimd.tensor_copy(xsrc[:], xin[:])
                yield
                for ktg in range(KT // 4):
                    tp = mm_ps.tile([128, 512], F32, tag="mm", name="tp")
                    tpv = tp[:].bitcast(MMDT)
                    for j in range(4):
                        kt = 4 * ktg + j
                        nc.tensor.transpose(tpv[:, j * 128:(j + 1) * 128],
                                            xsrc[:, kt * 128:(kt + 1) * 128], ident[:])
                    dst = xT[:].rearrange("p (kt t) -> p kt t", kt=KT)[
                        :, 4 * ktg:4 * ktg + 4, tt * 128:(tt + 1) * 128]
                    nc.vector.tensor_copy(
                        dst, tpv[:, 0:512].rearrange("p (j c) -> p j c", j=4))
                    yield
            qkv = []
            for w_sb, pool, tag in ((wq_sb, qT_p, "qT"), (wk_sb, kT_p, "kT"),
                                    (wv_sb, vT_p, "vT")):
                dest = pool.tile([128, T], MMDT, tag=tag, name=tag)
                qkv.append(dest)
                for ch in range(NCHUNK):
                    pp = mm_ps.tile([128, 512], F32, tag="mm", name="pp")
                    for kt in range(KT):
                        nc.tensor.matmul(
                            pp[:], w_sb[:, kt * 128:(kt + 1) * 128],
                            xT[:, kt * T + ch * 512: kt * T + (ch + 1) * 512],
                            start=(kt == 0), stop=(kt == KT - 1))
                    nc.vector.tensor_copy(dest[:, ch * 512:(ch + 1) * 512], pp[:])
                    yield
            qT, kT_t, vT = qkv
            vaug = vaug_p.tile([128, NT * 130], MMDT, tag="vaug", name="vaug")
            for jtg in range(NT // 4):
                tp = mm_ps.tile([128, 512], F32, tag="mm", name="tpv")
                tpv = tp[:].bitcast(MMDT)
                for j in range(4):
                    jt = 4 * jtg + j
                    nc.tensor.transpose(tpv[:, j * 128:(j + 1) * 128],
                                        vT[:, jt * 128:(jt + 1) * 128], ident[:])
                vv = vaug[:].rearrange("p (jt c) -> p jt c", c=130)
                src = tpv[:, 0:512].rearrange("p (j c) -> p j c", j=4)
                nc.vector.tensor_copy(vv[:, 4 * jtg:4 * jtg + 4, 0:64], src[:, :, 0:64])
                nc.vector.tensor_copy(vv[:, 4 * jtg:4 * jtg + 4, 65:129], src[:, :, 64:128])
                yield
            nc.vector.tensor_copy(
                vaug[:].rearrange("p (u c) -> p u c", c=65)[:, :, 64:65],
                ones32[:].rearrange("p (u o) -> p u o", o=1))
            state[b] = (qT, kT_t, vaug)

        def phase34_steps(b):
            """Attention + chunk-wise output projection for batch b."""
            t0 = b * T
            qT, kT_t, vaug = state.pop(b)
            outT = outT_p.tile([128, T], MMDT, tag="outT", name="outT")
            for c in range(NCHUNK):
                njt = 4 * (c + 1)
                avp = {h: av_ps_p.tile([65, 512], F32, tag="av", name=f"avp{h}")
                       for h in (0, 1)}
                for jt in range(njt):
                    istart = max(512 * c, jt * 128)
                    off = istart - 512 * c
                    diag = jt >= 4 * c
                    dps, ats = {}, {}
                    for h in (0, 1):
                        dp = dots_ps.tile([128, 512], F32, tag="dots", name=f"dp{h}")
                        dps[h] = dp
                        nc.tensor.matmul(
                            dp[:, off:512],
                            kT_t[64 * h:64 * h + 64, jt * 128:(jt + 1) * 128],
                            qT[64 * h:64 * h + 64, istart:512 * (c + 1)],
                            start=True, stop=not diag)
                        if diag:
                            nc.tensor.matmul(
                                dp[:, off:off + 128], maskT[:], ident[:],
                                start=False, stop=True)
                    for h in (0, 1):
                        at = attnT_p.tile([128, 512], MMDT, tag="at", name=f"at{h}")
                        nc.scalar.activation(at[:, off:512], dps[h][:, off:512],
                                             mybir.ActivationFunctionType.Exp,
                                             bias=0.0, scale=float(SCALE))
                        ats[h] = at
                    for h in (0, 1):
                        nc.tensor.matmul(
                            avp[h][:, off:512],
                            vaug[:, jt * 130 + 65 * h: jt * 130 + 65 * h + 65],
                            ats[h][:, off:512],
                            start=(jt == 0), stop=(jt == njt - 1))
                    yield
                for h in (0, 1):
                    rc = recip_p.tile([1, 512], F32, tag="recip", name="rc")
                    nc.vector.reciprocal(rc[:], avp[h][64:65, :])
                    rb = rbc_p.tile([64, 512], F32, tag="rbc", name="rb")
                    nc.gpsimd.partition_broadcast(rb[:], rc[:])
                    nc.vector.scalar_tensor_tensor(
                        outT[64 * h:64 * h + 64, c * 512:(c + 1) * 512],
                        avp[h][0:64, :], 1.0, rb[:],
                        op0=mybir.AluOpType.mult, op1=mybir.AluOpType.mult)
                yield
                # output projection for the 4 token-tiles of this chunk
                for tt in range(4 * c, 4 * c + 4):
                    osb = osb_p.tile([128, DIM], F32, tag="osb", name="osb")
                    for half in (0, 1):
                        po = mm_ps.tile([128, 512], F32, tag="mm", name="po")
                        nc.tensor.matmul(po[:], outT[:, tt * 128:(tt + 1) * 128],
                                         wo_sb[:, half * 512:(half + 1) * 512],
                                         start=True, stop=True)
                        nc.vector.tensor_copy(osb[:, half * 512:(half + 1) * 512], po[:])
                    nc.scalar.dma_start(out_d[t0 + tt * 128: t0 + (tt + 1) * 128, :],
                                        osb[:])
                    yield

        def drive(gens):
            """Round-robin the emission generators until all are exhausted."""
            gens = [g for g in gens if g is not None]
            while gens:
                nxt = []
                for g in gens:
                    try:
                        next(g)
                        nxt.append(g)
                    except StopIteration:
                        pass
                gens = nxt

        rep_ctx = tc.For_i(0, repeat, 1) if repeat is not None else nullcontext()
        with rep_ctx:
            if interleave:
                for b in range(NB + 1):
                    drive([phase12_steps(b) if b < NB else None,
                           phase34_steps(b - 1) if b >= 1 else None])
            else:
                for b in range(NB):
                    drive([phase12_steps(b)])
                    drive([phase34_steps(b)])

    nc.compile()
    return nc


def kernel(x, Wq, Wkv, Wout, bout):
    """Full inputs -> full output. Shards across 8 NeuronCores internally."""
    if "nc" not in _CACHED:
        _CACHED["nc"] = build_kernel()
    nc = _CACHED["nc"]

    x = np.ascontiguousarray(np.asarray(x, dtype=np.float32).reshape(B * T, DIM))
    Wq = np.asarray(Wq, dtype=np.float32)
    Wkv = np.asarray(Wkv, dtype=np.float32)
    Wout = np.asarray(Wout, dtype=np.float32)
    bout = np.asarray(bout, dtype=np.float32)

    in_maps = []
    for c in range(8):
        s = slice(128 * c, 128 * (c + 1))
        in_maps.append({
            "x": x,
            "wq": np.ascontiguousarray(Wq[:, s]),
            "wk": np.ascontiguousarray(Wkv[:, :DIM][:, s]),
            "wv": np.ascontiguousarray(Wkv[:, DIM:][:, s]),
            "wo": np.ascontiguousarray(Wout[s, :]),
        })

    res = bass_utils.run_bass_kernel_spmd(nc, in_maps, core_ids=list(range(8)))
    acc = res.results[0]["out"].astype(np.float64)
    for c in range(1, 8):
        acc += res.results[c]["out"]
    out = (acc + bout.astype(np.float64)).astype(np.float32)
    return out.reshape(B, T, DIM)


# revision 13
# speedup vs baseline: 1.3858x; 1.0296x over previous
"""Causal multi-head attention block (16 heads, dim 1024) on 8 TRN2 NeuronCores.

Sharding: tensor-parallel over heads — core c computes heads {2c, 2c+1}:
  q/k/v projections with the 128-column weight slices, causal attention,
  and a partial output projection with the matching 128 Wout rows.
Host sums the 8 partial outputs and adds the bias.

Per-core dataflow (per batch of 2048 tokens):
  phase12: x tiles -> PE-transpose -> xT (dim-major); qT/kT/vT = W.T @ xT
           (feature-major, 2 heads packed on 128 partitions); vT ->
           PE-transpose -> v_aug (tok-major, 65 cols/head: 64 v + ones).
  phase34: scores TRANSPOSED: dotsT[j,i] = kT.T @ qT, causal mask added
           via a second matmul in the same accumulation group, exp on ACT
           (no max-subtraction needed; exponents are small) -> attnT;
           AV: outT = v_aug.T @ attnT accumulated over j-tiles; psum row 64
           = softmax denominators (ones-column trick). Normalize, then
           output projection per 128-token tile, DMA out.

Engines run their instruction streams IN ORDER, so phase12(b+1) emission is
interleaved with phase34(b) to fill PE gaps left by exp latency and to keep
DMA/DVE/ACT busy concurrently (software pipelining at emission order).
"""
import numpy as np
from contextlib import ExitStack, nullcontext

import concourse.bacc as bacc
import concourse.mybir as mybir
import concourse.tile as tile
import concourse.bass_utils as bass_utils
from concourse import masks

F32 = mybir.dt.float32
F32R = mybir.dt.float32r
BF16 = mybir.dt.bfloat16

B = 4            # batches
T = 2048         # tokens per batch
DIM = 1024
NT = T // 128    # token tiles per batch (16)
KT = DIM // 128  # contraction tiles (8)
NCHUNK = T // 512  # 512-col i-chunks per batch (4)
SCALE = DIM ** -0.5  # 1/32 — NOTE: full dim, not head dim (matches reference)
MASK_NEG = -1.0e9

DEFAULT_MMDT = "bf16"
_CACHED = {}


def build_kernel(repeat=None, mmdt=None, nbatches=None, interleave=True):
    mmdt = mmdt or DEFAULT_MMDT
    MMDT = {"f32r": F32R, "bf16": BF16}[mmdt]
    NB = nbatches if nbatches is not None else B

    nc = bacc.Bacc("TRN2", target_bir_lowering=False, debug=False, num_devices=8)

    xdt = F32R if MMDT == F32R else F32
    x_d = nc.dram_tensor("x", [B * T, DIM], xdt, kind="ExternalInput").ap()
    wq_d = nc.dram_tensor("wq", [DIM, 128], xdt, kind="ExternalInput").ap()
    wk_d = nc.dram_tensor("wk", [DIM, 128], xdt, kind="ExternalInput").ap()
    wv_d = nc.dram_tensor("wv", [DIM, 128], xdt, kind="ExternalInput").ap()
    wo_d = nc.dram_tensor("wo", [128, DIM], xdt, kind="ExternalInput").ap()
    out_d = nc.dram_tensor("out", [B * T, DIM], F32, kind="ExternalOutput").ap()

    with tile.TileContext(nc) as tc, ExitStack() as ctx:
        cp = ctx.enter_context(tc.tile_pool(name="const", bufs=1))
        xin_p = ctx.enter_context(tc.tile_pool(name="xin", bufs=4))
        xT_p = ctx.enter_context(tc.tile_pool(name="xT", bufs=2))
        qT_p = ctx.enter_context(tc.tile_pool(name="qT", bufs=2))
        kT_p = ctx.enter_context(tc.tile_pool(name="kT", bufs=2))
        vT_p = ctx.enter_context(tc.tile_pool(name="vT", bufs=2))
        vaug_p = ctx.enter_context(tc.tile_pool(name="vaug", bufs=2))
        attnT_p = ctx.enter_context(tc.tile_pool(name="attnT", bufs=6))
        recip_p = ctx.enter_context(tc.tile_pool(name="recip", bufs=2))
        rbc_p = ctx.enter_context(tc.tile_pool(name="rbc", bufs=2))
        outT_p = ctx.enter_context(tc.tile_pool(name="outT", bufs=2))
        osb_p = ctx.enter_context(tc.tile_pool(name="osb", bufs=3))
        mm_ps = ctx.enter_context(tc.tile_pool(name="mmps", bufs=2, space="PSUM"))
        dots_ps = ctx.enter_context(tc.tile_pool(name="dotsps", bufs=2, space="PSUM"))
        av_ps_p = ctx.enter_context(tc.tile_pool(name="avps", bufs=2, space="PSUM"))

        # ---- constants ----
        ident32 = cp.tile([128, 128], F32, tag="ident32")
        masks.make_identity(nc, ident32[:])
        ident = cp.tile([128, 128], MMDT, tag="ident")
        nc.vector.tensor_copy(ident[:], ident32[:])

        # maskT[i, j] = MASK_NEG where j > i (applied to dotsT via maskT.T @ I)
        maskT32 = cp.tile([128, 128], F32, tag="maskT32")
        nc.gpsimd.memset(maskT32[:], 0.0)
        nc.gpsimd.affine_select(
            out=maskT32[:], in_=maskT32[:],
            compare_op=mybir.AluOpType.is_ge, fill=MASK_NEG,
            base=0, pattern=[[-1, 128]], channel_multiplier=1,
        )
        maskT = cp.tile([128, 128], MMDT, tag="maskT")
        nc.vector.tensor_copy(maskT[:], maskT32[:])

        ones32 = cp.tile([128, 2 * NT], F32, tag="ones32")
        nc.gpsimd.memset(ones32[:], 1.0)

        # ---- weights ----
        wq_sb = cp.tile([128, KT * 128], MMDT, tag="wq")
        wk_sb = cp.tile([128, KT * 128], MMDT, tag="wk")
        wv_sb = cp.tile([128, KT * 128], MMDT, tag="wv")
        wo_sb = cp.tile([128, DIM], MMDT, tag="wo")
        if MMDT == F32R:
            for w_sb, w_d in ((wq_sb, wq_d), (wk_sb, wk_d), (wv_sb, wv_d)):
                nc.sync.dma_start(w_sb[:].rearrange("p (kt m) -> p kt m", kt=KT),
                                  w_d.rearrange("(kt p) m -> p kt m", p=128))
            nc.sync.dma_start(wo_sb[:], wo_d)
        else:
            for w_sb, w_d, wtag in ((wq_sb, wq_d, "q"), (wk_sb, wk_d, "k"),
                                    (wv_sb, wv_d, "v")):
                w32 = cp.tile([128, KT * 128], F32, tag=f"w32{wtag}", name=f"w32{wtag}")
                nc.sync.dma_start(w32[:].rearrange("p (kt m) -> p kt m", kt=KT),
                                  w_d.rearrange("(kt p) m -> p kt m", p=128))
                nc.vector.tensor_copy(w_sb[:], w32[:])
            wo32 = cp.tile([128, DIM], F32, tag="wo32")
            nc.sync.dma_start(wo32[:], wo_d)
            nc.vector.tensor_copy(wo_sb[:], wo32[:])

        state = {}  # per-batch qT/kT/vaug handles

        def phase12_steps(b):
            """xT + projections + v_aug for batch b. Yields between steps."""
            t0 = b * T
            xT = xT_p.tile([128, KT * T], MMDT, tag="xT", name="xT")
            for tt in range(NT):
                xin = xin_p.tile([128, DIM], xdt, tag="xin", name="xin")
                nc.sync.dma_start(xin[:], x_d[t0 + tt * 128: t0 + (tt + 1) * 128, :])
                if MMDT == F32R:
                    xsrc = xin
                else:
                    xsrc = xin_p.tile([128, DIM], BF16, tag="xinb", name="xinb")
                    nc.gpsimd.tensor_copy(xsrc[:], xin[:])
                yield
                for ktg in range(KT // 4):
                    tp = mm_ps.tile([128, 512], F32, tag="mm", name="tp")
                    tpv = tp[:].bitcast(MMDT)
                    for j in range(4):
                        kt = 4 * ktg + j
                        nc.tensor.transpose(tpv[:, j * 128:(j + 1) * 128],
                                            xsrc[:, kt * 128:(kt + 1) * 128], ident[:])
                    dst = xT[:].rearrange("p (kt t) -> p kt t", kt=KT)[
                        :, 4 * ktg:4 * ktg + 4, tt * 128:(tt + 1) * 128]
                    nc.vector.tensor_copy(
                        dst, tpv[:, 0:512].rearrange("p (j c) -> p j c", j=4))
                    yield
            qkv = []
            for w_sb, pool, tag in ((wq_sb, qT_p, "qT"), (wk_sb, kT_p, "kT"),
                                    (wv_sb, vT_p, "vT")):
                dest = pool.tile([128, T], MMDT, tag=tag, name=tag)
                qkv.append(dest)
                for ch in range(NCHUNK):
                    pp = mm_ps.tile([128, 512], F32, tag="mm", name="pp")
                    for kt in range(KT):
                        nc.tensor.matmul(
                            pp[:], w_sb[:, kt * 128:(kt + 1) * 128],
                            xT[:, kt * T + ch * 512: kt * T + (ch + 1) * 512],
                            start=(kt == 0), stop=(kt == KT - 1))
                    nc.vector.tensor_copy(dest[:, ch * 512:(ch + 1) * 512], pp[:])
                    yield
            qT, kT_t, vT = qkv
            vaug = vaug_p.tile([128, NT * 130], MMDT, tag="vaug", name="vaug")
            for jtg in range(NT // 4):
                tp = mm_ps.tile([128, 512], F32, tag="mm", name="tpv")
                tpv = tp[:].bitcast(MMDT)
                for j in range(4):
                    jt = 4 * jtg + j
                    nc.tensor.transpose(tpv[:, j * 128:(j + 1) * 128],
                                        vT[:, jt * 128:(jt + 1) * 128], ident[:])
                vv = vaug[:].rearrange("p (jt c) -> p jt c", c=130)
                src = tpv[:, 0:512].rearrange("p (j c) -> p j c", j=4)
                nc.vector.tensor_copy(vv[:, 4 * jtg:4 * jtg + 4, 0:64], src[:, :, 0:64])
                nc.vector.tensor_copy(vv[:, 4 * jtg:4 * jtg + 4, 65:129], src[:, :, 64:128])
                yield
            nc.vector.tensor_copy(
                vaug[:].rearrange("p (u c) -> p u c", c=65)[:, :, 64:65],
                ones32[:].rearrange("p (u o) -> p u o", o=1))
            state[b] = (qT, kT_t, vaug)

        def phase34_steps(b):
            """Attention + chunk-wise output projection for batch b."""
            t0 = b * T
            qT, kT_t, vaug = state.pop(b)
            outT = outT_p.tile([128, T], MMDT, tag="outT", name="outT")
            for c in range(NCHUNK):
                njt = 4 * (c + 1)
                avp = {h: av_ps_p.tile([65, 512], F32, tag="av", name=f"avp{h}")
                       for h in (0, 1)}
                for jp in range(njt // 2):
                    jts = (2 * jp, 2 * jp + 1)
                    offs = [max(512 * c, jt * 128) - 512 * c for jt in jts]
                    dps, ats = {}, {}
                    for h in (0, 1):
                        dp = dots_ps.tile([128, 1024], F32, tag="dots", name=f"dp{h}")
                        dps[h] = dp
                        for j, jt in enumerate(jts):
                            diag = jt >= 4 * c
                            off = offs[j]
                            nc.tensor.matmul(
                                dp[:, j * 512 + off: (j + 1) * 512],
                                kT_t[64 * h:64 * h + 64, jt * 128:(jt + 1) * 128],
                                qT[64 * h:64 * h + 64, 512 * c + off:512 * (c + 1)],
                                start=True, stop=not diag)
                            if diag:
                                nc.tensor.matmul(
                                    dp[:, j * 512 + off: j * 512 + off + 128],
                                    maskT[:], ident[:], start=False, stop=True)
                    for h in (0, 1):
                        at = attnT_p.tile([128, 1024], MMDT, tag="at", name=f"at{h}")
                        nc.scalar.activation(at[:, offs[0]:1024], dps[h][:, offs[0]:1024],
                                             mybir.ActivationFunctionType.Exp,
                                             bias=0.0, scale=float(SCALE))
                        ats[h] = at
                    for h in (0, 1):
                        for j, jt in enumerate(jts):
                            off = offs[j]
                            nc.tensor.matmul(
                                avp[h][:, off:512],
                                vaug[:, jt * 130 + 65 * h: jt * 130 + 65 * h + 65],
                                ats[h][:, j * 512 + off: (j + 1) * 512],
                                start=(jt == 0), stop=(jt == njt - 1))
                    yield
                for h in (0, 1):
                    rc = recip_p.tile([1, 512], F32, tag="recip", name="rc")
                    nc.vector.reciprocal(rc[:], avp[h][64:65, :])
                    rb = rbc_p.tile([64, 512], F32, tag="rbc", name="rb")
                    nc.gpsimd.partition_broadcast(rb[:], rc[:])
                    nc.vector.scalar_tensor_tensor(
                        outT[64 * h:64 * h + 64, c * 512:(c + 1) * 512],
                        avp[h][0:64, :], 1.0, rb[:],
                        op0=mybir.AluOpType.mult, op1=mybir.AluOpType.mult)
                yield
                # output projection for the 4 token-tiles of this chunk
                for tt in range(4 * c, 4 * c + 4):
                    osb = osb_p.tile([128, DIM], F32, tag="osb", name="osb")
                    for half in (0, 1):
                        po = mm_ps.tile([128, 512], F32, tag="mm", name="po")
                        nc.tensor.matmul(po[:], outT[:, tt * 128:(tt + 1) * 128],
                                         wo_sb[:, half * 512:(half + 1) * 512],
                                         start=True, stop=True)
                        nc.vector.tensor_copy(osb[:, half * 512:(half + 1) * 512], po[:])
                    nc.scalar.dma_start(out_d[t0 + tt * 128: t0 + (tt + 1) * 128, :],
                                        osb[:])
                    yield

        def drive(gens):
            """Round-robin the emission generators until all are exhausted."""
            gens = [g for g in gens if g is not None]
            while gens:
                nxt = []
                for g in gens:
                    try:
                        next(g)
                        nxt.append(g)
                    except StopIteration:
                        pass
                gens = nxt

        rep_ctx = tc.For_i(0, repeat, 1) if repeat is not None else nullcontext()
        with rep_ctx:
            if interleave:
                for b in range(NB + 1):
                    drive([phase12_steps(b) if b < NB else None,
                           phase34_steps(b - 1) if b >= 1 else None])
            else:
                for b in range(NB):
                    drive([phase12_steps(b)])
                    drive([phase34_steps(b)])

    nc.compile()
    return nc


def kernel(x, Wq, Wkv, Wout, bout):
    """Full inputs -> full output. Shards across 8 NeuronCores internally."""
    if "nc" not in _CACHED:
        _CACHED["nc"] = build_kernel()
    nc = _CACHED["nc"]

    x = np.ascontiguousarray(np.asarray(x, dtype=np.float32).reshape(B * T, DIM))
    Wq = np.asarray(Wq, dtype=np.float32)
    Wkv = np.asarray(Wkv, dtype=np.float32)
    Wout = np.asarray(Wout, dtype=np.float32)
    bout = np.asarray(bout, dtype=np.float32)

    in_maps = []
    for c in range(8):
        s = slice(128 * c, 128 * (c + 1))
        in_maps.append({
            "x": x,
            "wq": np.ascontiguousarray(Wq[:, s]),
            "wk": np.ascontiguousarray(Wkv[:, :DIM][:, s]),
            "wv": np.ascontiguousarray(Wkv[:, DIM:][:, s]),
            "wo": np.ascontiguousarray(Wout[s, :]),
        })

    res = bass_utils.run_bass_kernel_spmd(nc, in_maps, core_ids=list(range(8)))
    acc = res.results[0]["out"].astype(np.float64)
    for c in range(1, 8):
        acc += res.results[c]["out"]
    out = (acc + bout.astype(np.float64)).astype(np.float32)
    return out.reshape(B, T, DIM)


# revision 14
# speedup vs baseline: 1.4363x; 1.0365x over previous
"""Causal multi-head attention block (16 heads, dim 1024) on 8 TRN2 NeuronCores.

Sharding: tensor-parallel over heads — core c computes heads {2c, 2c+1}:
  q/k/v projections with the 128-column weight slices, causal attention,
  and a partial output projection with the matching 128 Wout rows.
Host sums the 8 partial outputs and adds the bias.

Per-core dataflow (per batch of 2048 tokens):
  phase12: x tiles -> PE-transpose -> xT (dim-major); qT/kT/vT = W.T @ xT
           (feature-major, 2 heads packed on 128 partitions); vT ->
           PE-transpose -> v_aug (tok-major, 65 cols/head: 64 v + ones).
  phase34: scores TRANSPOSED: dotsT[j,i] = kT.T @ qT, causal mask added
           via a second matmul in the same accumulation group, exp on ACT
           (no max-subtraction needed; exponents are small) -> attnT;
           AV: outT = v_aug.T @ attnT accumulated over j-tiles; psum row 64
           = softmax denominators (ones-column trick). Normalize, then
           output projection per 128-token tile, DMA out.

Engines run their instruction streams IN ORDER, so phase12(b+1) emission is
interleaved with phase34(b) to fill PE gaps left by exp latency and to keep
DMA/DVE/ACT busy concurrently (software pipelining at emission order).
"""
import numpy as np
from contextlib import ExitStack, nullcontext

import concourse.bacc as bacc
import concourse.mybir as mybir
import concourse.tile as tile
import concourse.bass_utils as bass_utils
from concourse import masks

F32 = mybir.dt.float32
F32R = mybir.dt.float32r
BF16 = mybir.dt.bfloat16

B = 4            # batches
T = 2048         # tokens per batch
DIM = 1024
NT = T // 128    # token tiles per batch (16)
KT = DIM // 128  # contraction tiles (8)
NCHUNK = T // 512  # 512-col i-chunks per batch (4)
SCALE = DIM ** -0.5  # 1/32 — NOTE: full dim, not head dim (matches reference)
MASK_NEG = -1.0e9

DEFAULT_MMDT = "bf16"
_CACHED = {}


def build_kernel(repeat=None, mmdt=None, nbatches=None, interleave=True):
    mmdt = mmdt or DEFAULT_MMDT
    MMDT = {"f32r": F32R, "bf16": BF16}[mmdt]
    NB = nbatches if nbatches is not None else B

    nc = bacc.Bacc("TRN2", target_bir_lowering=False, debug=False, num_devices=8)

    xdt = F32R if MMDT == F32R else F32
    x_d = nc.dram_tensor("x", [B * T, DIM], xdt, kind="ExternalInput").ap()
    wq_d = nc.dram_tensor("wq", [DIM, 128], xdt, kind="ExternalInput").ap()
    wk_d = nc.dram_tensor("wk", [DIM, 128], xdt, kind="ExternalInput").ap()
    wv_d = nc.dram_tensor("wv", [DIM, 128], xdt, kind="ExternalInput").ap()
    wo_d = nc.dram_tensor("wo", [128, DIM], xdt, kind="ExternalInput").ap()
    out_d = nc.dram_tensor("out", [B * T, DIM], F32, kind="ExternalOutput").ap()

    with tile.TileContext(nc) as tc, ExitStack() as ctx:
        cp = ctx.enter_context(tc.tile_pool(name="const", bufs=1))
        xin_p = ctx.enter_context(tc.tile_pool(name="xin", bufs=6))
        xT_p = ctx.enter_context(tc.tile_pool(name="xT", bufs=2))
        qT_p = ctx.enter_context(tc.tile_pool(name="qT", bufs=2))
        kT_p = ctx.enter_context(tc.tile_pool(name="kT", bufs=2))
        vT_p = ctx.enter_context(tc.tile_pool(name="vT", bufs=2))
        vaug_p = ctx.enter_context(tc.tile_pool(name="vaug", bufs=2))
        attnT_p = ctx.enter_context(tc.tile_pool(name="attnT", bufs=6))
        recip_p = ctx.enter_context(tc.tile_pool(name="recip", bufs=2))
        rbc_p = ctx.enter_context(tc.tile_pool(name="rbc", bufs=2))
        outT_p = ctx.enter_context(tc.tile_pool(name="outT", bufs=2))
        osb_p = ctx.enter_context(tc.tile_pool(name="osb", bufs=3))
        mm_ps = ctx.enter_context(tc.tile_pool(name="mmps", bufs=2, space="PSUM"))
        dots_ps = ctx.enter_context(tc.tile_pool(name="dotsps", bufs=2, space="PSUM"))
        av_ps_p = ctx.enter_context(tc.tile_pool(name="avps", bufs=2, space="PSUM"))

        # ---- constants ----
        ident32 = cp.tile([128, 128], F32, tag="ident32")
        masks.make_identity(nc, ident32[:])
        ident = cp.tile([128, 128], MMDT, tag="ident")
        nc.vector.tensor_copy(ident[:], ident32[:])

        # maskT[i, j] = MASK_NEG where j > i (applied to dotsT via maskT.T @ I)
        maskT32 = cp.tile([128, 128], F32, tag="maskT32")
        nc.gpsimd.memset(maskT32[:], 0.0)
        nc.gpsimd.affine_select(
            out=maskT32[:], in_=maskT32[:],
            compare_op=mybir.AluOpType.is_ge, fill=MASK_NEG,
            base=0, pattern=[[-1, 128]], channel_multiplier=1,
        )
        maskT = cp.tile([128, 128], MMDT, tag="maskT")
        nc.vector.tensor_copy(maskT[:], maskT32[:])

        ones32 = cp.tile([128, 2 * NT], F32, tag="ones32")
        nc.gpsimd.memset(ones32[:], 1.0)

        # ---- weights ----
        wq_sb = cp.tile([128, KT * 128], MMDT, tag="wq")
        wk_sb = cp.tile([128, KT * 128], MMDT, tag="wk")
        wv_sb = cp.tile([128, KT * 128], MMDT, tag="wv")
        wo_sb = cp.tile([128, DIM], MMDT, tag="wo")
        if MMDT == F32R:
            for w_sb, w_d in ((wq_sb, wq_d), (wk_sb, wk_d), (wv_sb, wv_d)):
                nc.sync.dma_start(w_sb[:].rearrange("p (kt m) -> p kt m", kt=KT),
                                  w_d.rearrange("(kt p) m -> p kt m", p=128))
            nc.sync.dma_start(wo_sb[:], wo_d)
        else:
            for w_sb, w_d, wtag in ((wq_sb, wq_d, "q"), (wk_sb, wk_d, "k"),
                                    (wv_sb, wv_d, "v")):
                w32 = cp.tile([128, KT * 128], F32, tag=f"w32{wtag}", name=f"w32{wtag}")
                nc.sync.dma_start(w32[:].rearrange("p (kt m) -> p kt m", kt=KT),
                                  w_d.rearrange("(kt p) m -> p kt m", p=128))
                nc.vector.tensor_copy(w_sb[:], w32[:])
            wo32 = cp.tile([128, DIM], F32, tag="wo32")
            nc.sync.dma_start(wo32[:], wo_d)
            nc.vector.tensor_copy(wo_sb[:], wo32[:])

        state = {}  # per-batch qT/kT/vaug handles

        def phase12_steps(b):
            """xT + projections + v_aug for batch b. Yields between steps."""
            t0 = b * T
            xT = xT_p.tile([128, KT * T], MMDT, tag="xT", name="xT")
            for tt in range(NT):
                xin = xin_p.tile([128, DIM], xdt, tag="xin", name="xin")
                nc.sync.dma_start(xin[:], x_d[t0 + tt * 128: t0 + (tt + 1) * 128, :])
                if MMDT == F32R:
                    xsrc = xin
                else:
                    xsrc = xin_p.tile([128, DIM], BF16, tag="xinb", name="xinb")
                    nc.gpsimd.tensor_copy(xsrc[:], xin[:])
                yield
                ntpg = 4 if MMDT == F32R else 8  # transposes per psum bank
                for ktg in range(KT // ntpg):
                    tp = mm_ps.tile([128, 512], F32, tag="mm", name="tp")
                    tpv = tp[:].bitcast(MMDT)
                    for j in range(ntpg):
                        kt = ntpg * ktg + j
                        nc.tensor.transpose(tpv[:, j * 128:(j + 1) * 128],
                                            xsrc[:, kt * 128:(kt + 1) * 128], ident[:])
                    dst = xT[:].rearrange("p (kt t) -> p kt t", kt=KT)[
                        :, ntpg * ktg:ntpg * (ktg + 1), tt * 128:(tt + 1) * 128]
                    nc.vector.tensor_copy(
                        dst, tpv[:, 0:ntpg * 128].rearrange("p (j c) -> p j c", j=ntpg))
                    yield
            qkv = []
            for w_sb, pool, tag in ((wq_sb, qT_p, "qT"), (wk_sb, kT_p, "kT"),
                                    (wv_sb, vT_p, "vT")):
                dest = pool.tile([128, T], MMDT, tag=tag, name=tag)
                qkv.append(dest)
                for ch in range(NCHUNK):
                    pp = mm_ps.tile([128, 512], F32, tag="mm", name="pp")
                    for kt in range(KT):
                        nc.tensor.matmul(
                            pp[:], w_sb[:, kt * 128:(kt + 1) * 128],
                            xT[:, kt * T + ch * 512: kt * T + (ch + 1) * 512],
                            start=(kt == 0), stop=(kt == KT - 1))
                    nc.vector.tensor_copy(dest[:, ch * 512:(ch + 1) * 512], pp[:])
                    yield
            qT, kT_t, vT = qkv
            vaug = vaug_p.tile([128, NT * 130], MMDT, tag="vaug", name="vaug")
            nvg = 4 if MMDT == F32R else 8
            for jtg in range(NT // nvg):
                tp = mm_ps.tile([128, 512], F32, tag="mm", name="tpv")
                tpv = tp[:].bitcast(MMDT)
                for j in range(nvg):
                    jt = nvg * jtg + j
                    nc.tensor.transpose(tpv[:, j * 128:(j + 1) * 128],
                                        vT[:, jt * 128:(jt + 1) * 128], ident[:])
                vv = vaug[:].rearrange("p (jt c) -> p jt c", c=130)
                src = tpv[:, 0:nvg * 128].rearrange("p (j c) -> p j c", j=nvg)
                nc.vector.tensor_copy(vv[:, nvg * jtg:nvg * (jtg + 1), 0:64], src[:, :, 0:64])
                nc.vector.tensor_copy(vv[:, nvg * jtg:nvg * (jtg + 1), 65:129], src[:, :, 64:128])
                yield
            nc.vector.tensor_copy(
                vaug[:].rearrange("p (u c) -> p u c", c=65)[:, :, 64:65],
                ones32[:].rearrange("p (u o) -> p u o", o=1))
            state[b] = (qT, kT_t, vaug)

        def phase34_steps(b):
            """Attention + chunk-wise output projection for batch b."""
            t0 = b * T
            qT, kT_t, vaug = state.pop(b)
            outT = outT_p.tile([128, T], MMDT, tag="outT", name="outT")
            for c in range(NCHUNK):
                njt = 4 * (c + 1)
                avp = {h: av_ps_p.tile([65, 512], F32, tag="av", name=f"avp{h}")
                       for h in (0, 1)}
                for jp in range(njt // 2):
                    jts = (2 * jp, 2 * jp + 1)
                    offs = [max(512 * c, jt * 128) - 512 * c for jt in jts]
                    dps, ats = {}, {}
                    for h in (0, 1):
                        dp = dots_ps.tile([128, 1024], F32, tag="dots", name=f"dp{h}")
                        dps[h] = dp
                        for j, jt in enumerate(jts):
                            diag = jt >= 4 * c
                            off = offs[j]
                            nc.tensor.matmul(
                                dp[:, j * 512 + off: (j + 1) * 512],
                                kT_t[64 * h:64 * h + 64, jt * 128:(jt + 1) * 128],
                                qT[64 * h:64 * h + 64, 512 * c + off:512 * (c + 1)],
                                start=True, stop=not diag)
                            if diag:
                                nc.tensor.matmul(
                                    dp[:, j * 512 + off: j * 512 + off + 128],
                                    maskT[:], ident[:], start=False, stop=True)
                    for h in (0, 1):
                        at = attnT_p.tile([128, 1024], MMDT, tag="at", name=f"at{h}")
                        nc.scalar.activation(at[:, offs[0]:1024], dps[h][:, offs[0]:1024],
                                             mybir.ActivationFunctionType.Exp,
                                             bias=0.0, scale=float(SCALE))
                        ats[h] = at
                    for h in (0, 1):
                        for j, jt in enumerate(jts):
                            off = offs[j]
                            nc.tensor.matmul(
                                avp[h][:, off:512],
                                vaug[:, jt * 130 + 65 * h: jt * 130 + 65 * h + 65],
                                ats[h][:, j * 512 + off: (j + 1) * 512],
                                start=(jt == 0), stop=(jt == njt - 1))
                    yield
                for h in (0, 1):
                    rc = recip_p.tile([1, 512], F32, tag="recip", name="rc")
                    nc.vector.reciprocal(rc[:], avp[h][64:65, :])
                    rb = rbc_p.tile([64, 512], F32, tag="rbc", name="rb")
                    nc.gpsimd.partition_broadcast(rb[:], rc[:])
                    nc.vector.scalar_tensor_tensor(
                        outT[64 * h:64 * h + 64, c * 512:(c + 1) * 512],
                        avp[h][0:64, :], 1.0, rb[:],
                        op0=mybir.AluOpType.mult, op1=mybir.AluOpType.mult)
                yield
                # output projection for the 4 token-tiles of this chunk
                for tt in range(4 * c, 4 * c + 4):
                    osb = osb_p.tile([128, DIM], F32, tag="osb", name="osb")
                    for half in (0, 1):
                        po = mm_ps.tile([128, 512], F32, tag="mm", name="po")
                        nc.tensor.matmul(po[:], outT[:, tt * 128:(tt + 1) * 128],
                                         wo_sb[:, half * 512:(half + 1) * 512],
                                         start=True, stop=True)
                        nc.vector.tensor_copy(osb[:, half * 512:(half + 1) * 512], po[:])
                    nc.scalar.dma_start(out_d[t0 + tt * 128: t0 + (tt + 1) * 128, :],
                                        osb[:])
                    yield

        def drive(gens):
            """Round-robin the emission generators until all are exhausted."""
            gens = [g for g in gens if g is not None]
            while gens:
                nxt = []
                for g in gens:
                    try:
                        next(g)
                        nxt.append(g)
                    except StopIteration:
                        pass
                gens = nxt

        rep_ctx = tc.For_i(0, repeat, 1) if repeat is not None else nullcontext()
        with rep_ctx:
            if interleave:
                for b in range(NB + 1):
                    drive([phase12_steps(b) if b < NB else None,
                           phase34_steps(b - 1) if b >= 1 else None])
            else:
                for b in range(NB):
                    drive([phase12_steps(b)])
                    drive([phase34_steps(b)])

    nc.compile()
    return nc


def kernel(x, Wq, Wkv, Wout, bout):
    """Full inputs -> full output. Shards across 8 NeuronCores internally."""
    if "nc" not in _CACHED:
        _CACHED["nc"] = build_kernel()
    nc = _CACHED["nc"]

    x = np.ascontiguousarray(np.asarray(x, dtype=np.float32).reshape(B * T, DIM))
    Wq = np.asarray(Wq, dtype=np.float32)
    Wkv = np.asarray(Wkv, dtype=np.float32)
    Wout = np.asarray(Wout, dtype=np.float32)
    bout = np.asarray(bout, dtype=np.float32)

    in_maps = []
    for c in range(8):
        s = slice(128 * c, 128 * (c + 1))
        in_maps.append({
            "x": x,
            "wq": np.ascontiguousarray(Wq[:, s]),
            "wk": np.ascontiguousarray(Wkv[:, :DIM][:, s]),
            "wv": np.ascontiguousarray(Wkv[:, DIM:][:, s]),
            "wo": np.ascontiguousarray(Wout[s, :]),
        })

    res = bass_utils.run_bass_kernel_spmd(nc, in_maps, core_ids=list(range(8)))
    acc = res.results[0]["out"].astype(np.float64)
    for c in range(1, 8):
        acc += res.results[c]["out"]
    out = (acc + bout.astype(np.float64)).astype(np.float32)
    return out.reshape(B, T, DIM)


# revision 15
# speedup vs baseline: 1.7381x; 1.2101x over previous
"""Causal multi-head attention block (16 heads, dim 1024) on 8 TRN2 NeuronCores.

Sharding: tensor-parallel over heads — core c computes heads {2c, 2c+1}:
  q/k/v projections with the 128-column weight slices, causal attention,
  and a partial output projection with the matching 128 Wout rows.
Host sums the 8 partial outputs and adds the bias.

Per-core dataflow (per batch of 2048 tokens):
  phase12: x tiles -> PE-transpose -> xT (dim-major); qT/kT/vT = W.T @ xT
           (feature-major, 2 heads packed on 128 partitions); vT ->
           PE-transpose -> v_aug (tok-major, 65 cols/head: 64 v + ones).
  phase34: scores TRANSPOSED: dotsT[j,i] = kT.T @ qT, causal mask added
           via a second matmul in the same accumulation group, exp on ACT
           (no max-subtraction needed; exponents are small) -> attnT;
           AV: outT = v_aug.T @ attnT accumulated over j-tiles; psum row 64
           = softmax denominators (ones-column trick). Normalize, then
           output projection per 128-token tile, DMA out.

Engines run their instruction streams IN ORDER, so phase12(b+1) emission is
interleaved with phase34(b) to fill PE gaps left by exp latency and to keep
DMA/DVE/ACT busy concurrently (software pipelining at emission order).
"""
import numpy as np
import ml_dtypes
from contextlib import ExitStack, nullcontext

import concourse.bacc as bacc
import concourse.mybir as mybir
import concourse.tile as tile
import concourse.bass_utils as bass_utils
from concourse import masks

F32 = mybir.dt.float32
F32R = mybir.dt.float32r
BF16 = mybir.dt.bfloat16
FP16 = mybir.dt.float16

B = 4            # batches
T = 2048         # tokens per batch
DIM = 1024
NT = T // 128    # token tiles per batch (16)
KT = DIM // 128  # contraction tiles (8)
NCHUNK = T // 512  # 512-col i-chunks per batch (4)
SCALE = DIM ** -0.5  # 1/32 — NOTE: full dim, not head dim (matches reference)
MASK_NEG = -1.0e9

DEFAULT_MMDT = "bf16"
_CACHED = {}


def build_kernel(repeat=None, mmdt=None, nbatches=None, interleave=True):
    mmdt = mmdt or DEFAULT_MMDT
    MMDT = {"f32r": F32R, "bf16": BF16}[mmdt]
    NB = nbatches if nbatches is not None else B

    nc = bacc.Bacc("TRN2", target_bir_lowering=False, debug=False, num_devices=8)

    xdt = F32R if MMDT == F32R else BF16
    odt = F32 if MMDT == F32R else FP16
    x_d = nc.dram_tensor("x", [B * T, DIM], xdt, kind="ExternalInput").ap()
    wq_d = nc.dram_tensor("wq", [DIM, 128], xdt, kind="ExternalInput").ap()
    wk_d = nc.dram_tensor("wk", [DIM, 128], xdt, kind="ExternalInput").ap()
    wv_d = nc.dram_tensor("wv", [DIM, 128], xdt, kind="ExternalInput").ap()
    wo_d = nc.dram_tensor("wo", [128, DIM], xdt, kind="ExternalInput").ap()
    out_d = nc.dram_tensor("out", [B * T, DIM], odt, kind="ExternalOutput").ap()

    with tile.TileContext(nc) as tc, ExitStack() as ctx:
        cp = ctx.enter_context(tc.tile_pool(name="const", bufs=1))
        xin_p = ctx.enter_context(tc.tile_pool(name="xin", bufs=6))
        xT_p = ctx.enter_context(tc.tile_pool(name="xT", bufs=2))
        qT_p = ctx.enter_context(tc.tile_pool(name="qT", bufs=2))
        kT_p = ctx.enter_context(tc.tile_pool(name="kT", bufs=2))
        vT_p = ctx.enter_context(tc.tile_pool(name="vT", bufs=2))
        vaug_p = ctx.enter_context(tc.tile_pool(name="vaug", bufs=2))
        attnT_p = ctx.enter_context(tc.tile_pool(name="attnT", bufs=6))
        recip_p = ctx.enter_context(tc.tile_pool(name="recip", bufs=2))
        rbc_p = ctx.enter_context(tc.tile_pool(name="rbc", bufs=2))
        outT_p = ctx.enter_context(tc.tile_pool(name="outT", bufs=2))
        osb_p = ctx.enter_context(tc.tile_pool(name="osb", bufs=3))
        mm_ps = ctx.enter_context(tc.tile_pool(name="mmps", bufs=2, space="PSUM"))
        dots_ps = ctx.enter_context(tc.tile_pool(name="dotsps", bufs=2, space="PSUM"))
        av_ps_p = ctx.enter_context(tc.tile_pool(name="avps", bufs=2, space="PSUM"))

        # ---- constants ----
        ident32 = cp.tile([128, 128], F32, tag="ident32")
        masks.make_identity(nc, ident32[:])
        ident = cp.tile([128, 128], MMDT, tag="ident")
        nc.vector.tensor_copy(ident[:], ident32[:])

        # maskT[i, j] = MASK_NEG where j > i (applied to dotsT via maskT.T @ I)
        maskT32 = cp.tile([128, 128], F32, tag="maskT32")
        nc.gpsimd.memset(maskT32[:], 0.0)
        nc.gpsimd.affine_select(
            out=maskT32[:], in_=maskT32[:],
            compare_op=mybir.AluOpType.is_ge, fill=MASK_NEG,
            base=0, pattern=[[-1, 128]], channel_multiplier=1,
        )
        maskT = cp.tile([128, 128], MMDT, tag="maskT")
        nc.vector.tensor_copy(maskT[:], maskT32[:])

        ones32 = cp.tile([128, 2 * NT], F32, tag="ones32")
        nc.gpsimd.memset(ones32[:], 1.0)

        # ---- weights ----
        wq_sb = cp.tile([128, KT * 128], MMDT, tag="wq")
        wk_sb = cp.tile([128, KT * 128], MMDT, tag="wk")
        wv_sb = cp.tile([128, KT * 128], MMDT, tag="wv")
        wo_sb = cp.tile([128, DIM], MMDT, tag="wo")
        for w_sb, w_d in ((wq_sb, wq_d), (wk_sb, wk_d), (wv_sb, wv_d)):
            nc.sync.dma_start(w_sb[:].rearrange("p (kt m) -> p kt m", kt=KT),
                              w_d.rearrange("(kt p) m -> p kt m", p=128))
        nc.sync.dma_start(wo_sb[:], wo_d)

        state = {}  # per-batch qT/kT/vaug handles

        def phase12_steps(b):
            """xT + projections + v_aug for batch b. Yields between steps."""
            t0 = b * T
            xT = xT_p.tile([128, KT * T], MMDT, tag="xT", name="xT")
            for tt in range(NT):
                xin = xin_p.tile([128, DIM], xdt, tag="xin", name="xin")
                nc.sync.dma_start(xin[:], x_d[t0 + tt * 128: t0 + (tt + 1) * 128, :])
                xsrc = xin
                yield
                ntpg = 4 if MMDT == F32R else 8  # transposes per psum bank
                for ktg in range(KT // ntpg):
                    tp = mm_ps.tile([128, 512], F32, tag="mm", name="tp")
                    tpv = tp[:].bitcast(MMDT)
                    for j in range(ntpg):
                        kt = ntpg * ktg + j
                        nc.tensor.transpose(tpv[:, j * 128:(j + 1) * 128],
                                            xsrc[:, kt * 128:(kt + 1) * 128], ident[:])
                    dst = xT[:].rearrange("p (kt t) -> p kt t", kt=KT)[
                        :, ntpg * ktg:ntpg * (ktg + 1), tt * 128:(tt + 1) * 128]
                    nc.vector.tensor_copy(
                        dst, tpv[:, 0:ntpg * 128].rearrange("p (j c) -> p j c", j=ntpg))
                    yield
            qkv = []
            for w_sb, pool, tag in ((wq_sb, qT_p, "qT"), (wk_sb, kT_p, "kT"),
                                    (wv_sb, vT_p, "vT")):
                dest = pool.tile([128, T], MMDT, tag=tag, name=tag)
                qkv.append(dest)
                for ch in range(NCHUNK):
                    pp = mm_ps.tile([128, 512], F32, tag="mm", name="pp")
                    for kt in range(KT):
                        nc.tensor.matmul(
                            pp[:], w_sb[:, kt * 128:(kt + 1) * 128],
                            xT[:, kt * T + ch * 512: kt * T + (ch + 1) * 512],
                            start=(kt == 0), stop=(kt == KT - 1))
                    nc.vector.tensor_copy(dest[:, ch * 512:(ch + 1) * 512], pp[:])
                    yield
            qT, kT_t, vT = qkv
            vaug = vaug_p.tile([128, NT * 130], MMDT, tag="vaug", name="vaug")
            nvg = 4 if MMDT == F32R else 8
            for jtg in range(NT // nvg):
                tp = mm_ps.tile([128, 512], F32, tag="mm", name="tpv")
                tpv = tp[:].bitcast(MMDT)
                for j in range(nvg):
                    jt = nvg * jtg + j
                    nc.tensor.transpose(tpv[:, j * 128:(j + 1) * 128],
                                        vT[:, jt * 128:(jt + 1) * 128], ident[:])
                vv = vaug[:].rearrange("p (jt c) -> p jt c", c=130)
                src = tpv[:, 0:nvg * 128].rearrange("p (j c) -> p j c", j=nvg)
                nc.vector.tensor_copy(vv[:, nvg * jtg:nvg * (jtg + 1), 0:64], src[:, :, 0:64])
                nc.vector.tensor_copy(vv[:, nvg * jtg:nvg * (jtg + 1), 65:129], src[:, :, 64:128])
                yield
            nc.vector.tensor_copy(
                vaug[:].rearrange("p (u c) -> p u c", c=65)[:, :, 64:65],
                ones32[:].rearrange("p (u o) -> p u o", o=1))
            state[b] = (qT, kT_t, vaug)

        def phase34_steps(b):
            """Attention + chunk-wise output projection for batch b."""
            t0 = b * T
            qT, kT_t, vaug = state.pop(b)
            outT = outT_p.tile([128, T], MMDT, tag="outT", name="outT")
            for c in range(NCHUNK):
                njt = 4 * (c + 1)
                avp = {h: av_ps_p.tile([65, 512], F32, tag="av", name=f"avp{h}")
                       for h in (0, 1)}
                for jp in range(njt // 2):
                    jts = (2 * jp, 2 * jp + 1)
                    offs = [max(512 * c, jt * 128) - 512 * c for jt in jts]
                    dps, ats = {}, {}
                    for h in (0, 1):
                        dp = dots_ps.tile([128, 1024], F32, tag="dots", name=f"dp{h}")
                        dps[h] = dp
                        for j, jt in enumerate(jts):
                            diag = jt >= 4 * c
                            off = offs[j]
                            nc.tensor.matmul(
                                dp[:, j * 512 + off: (j + 1) * 512],
                                kT_t[64 * h:64 * h + 64, jt * 128:(jt + 1) * 128],
                                qT[64 * h:64 * h + 64, 512 * c + off:512 * (c + 1)],
                                start=True, stop=not diag)
                            if diag:
                                nc.tensor.matmul(
                                    dp[:, j * 512 + off: j * 512 + off + 128],
                                    maskT[:], ident[:], start=False, stop=True)
                    for h in (0, 1):
                        at = attnT_p.tile([128, 1024], MMDT, tag="at", name=f"at{h}")
                        nc.scalar.activation(at[:, offs[0]:1024], dps[h][:, offs[0]:1024],
                                             mybir.ActivationFunctionType.Exp,
                                             bias=0.0, scale=float(SCALE))
                        ats[h] = at
                    for h in (0, 1):
                        for j, jt in enumerate(jts):
                            off = offs[j]
                            nc.tensor.matmul(
                                avp[h][:, off:512],
                                vaug[:, jt * 130 + 65 * h: jt * 130 + 65 * h + 65],
                                ats[h][:, j * 512 + off: (j + 1) * 512],
                                start=(jt == 0), stop=(jt == njt - 1))
                    yield
                for h in (0, 1):
                    rc = recip_p.tile([1, 512], F32, tag="recip", name="rc")
                    nc.vector.reciprocal(rc[:], avp[h][64:65, :])
                    rb = rbc_p.tile([64, 512], F32, tag="rbc", name="rb")
                    nc.gpsimd.partition_broadcast(rb[:], rc[:])
                    nc.vector.scalar_tensor_tensor(
                        outT[64 * h:64 * h + 64, c * 512:(c + 1) * 512],
                        avp[h][0:64, :], 1.0, rb[:],
                        op0=mybir.AluOpType.mult, op1=mybir.AluOpType.mult)
                yield
                # output projection for the 4 token-tiles of this chunk
                for tt in range(4 * c, 4 * c + 4):
                    osb = osb_p.tile([128, DIM], odt, tag="osb", name="osb")
                    for half in (0, 1):
                        po = mm_ps.tile([128, 512], F32, tag="mm", name="po")
                        nc.tensor.matmul(po[:], outT[:, tt * 128:(tt + 1) * 128],
                                         wo_sb[:, half * 512:(half + 1) * 512],
                                         start=True, stop=True)
                        nc.vector.tensor_copy(osb[:, half * 512:(half + 1) * 512], po[:])
                    nc.scalar.dma_start(out_d[t0 + tt * 128: t0 + (tt + 1) * 128, :],
                                        osb[:])
                    yield

        def drive(gens):
            """Round-robin the emission generators until all are exhausted."""
            gens = [g for g in gens if g is not None]
            while gens:
                nxt = []
                for g in gens:
                    try:
                        next(g)
                        nxt.append(g)
                    except StopIteration:
                        pass
                gens = nxt

        rep_ctx = tc.For_i(0, repeat, 1) if repeat is not None else nullcontext()
        with rep_ctx:
            if interleave:
                for b in range(NB + 1):
                    drive([phase12_steps(b) if b < NB else None,
                           phase34_steps(b - 1) if b >= 1 else None])
            else:
                for b in range(NB):
                    drive([phase12_steps(b)])
                    drive([phase34_steps(b)])

    nc.compile()
    return nc


def kernel(x, Wq, Wkv, Wout, bout):
    """Full inputs -> full output. Shards across 8 NeuronCores internally."""
    if "nc" not in _CACHED:
        _CACHED["nc"] = build_kernel()
    nc = _CACHED["nc"]

    hdt = np.float32 if DEFAULT_MMDT == "f32r" else ml_dtypes.bfloat16
    x = np.ascontiguousarray(np.asarray(x, dtype=np.float32).reshape(B * T, DIM)).astype(hdt)
    Wq = np.asarray(Wq, dtype=np.float32).astype(hdt)
    Wkv = np.asarray(Wkv, dtype=np.float32).astype(hdt)
    Wout = np.asarray(Wout, dtype=np.float32).astype(hdt)
    bout = np.asarray(bout, dtype=np.float32)

    in_maps = []
    for c in range(8):
        s = slice(128 * c, 128 * (c + 1))
        in_maps.append({
            "x": x,
            "wq": np.ascontiguousarray(Wq[:, s]),
            "wk": np.ascontiguousarray(Wkv[:, :DIM][:, s]),
            "wv": np.ascontiguousarray(Wkv[:, DIM:][:, s]),
            "wo": np.ascontiguousarray(Wout[s, :]),
        })

    res = bass_utils.run_bass_kernel_spmd(nc, in_maps, core_ids=list(range(8)))
    acc = res.results[0]["out"].astype(np.float64)
    for c in range(1, 8):
        acc += res.results[c]["out"]
    out = (acc + bout.astype(np.float64)).astype(np.float32)
    return out.reshape(B, T, DIM)
